# revision 20
# baseline (speedup 1.0000x reference)
"""Trainium2 Bass kernel for nn_MessagePassing_22204980920652.

Math background (validated against the reference to fp64 exactness):

The reference runs 3 rounds of (vertex_conv -> edge_conv) with SHARED weights.
Inside vertex_conv, everything up to the LayerNorm depends only on `features`
(never on the running edge state), so the LN output E is identical in every
round; the `prev`-mixing is a scalar recurrence  c_{i+1} = (1+g)c_i + (1-g)
on the multiplier of E.  Inside edge_conv, the MHA result is computed but
UNUSED (dead code) and the LN input is c_i * X for a round-independent
X = (inc @ E) @ WprojEC.T.  LayerNorm of a positively scaled input only
differs through its eps term:  LN(cX) = (X-mu) / sqrt(var + eps/c^2) * g + b.
So the whole network collapses to:

  attn   = MHA_vc(features)                       (the only attention)
  S      = inc.T @ attn            [e, d]
  w      = softmax_e(S);  E = LN_vc((S*w) @ WprojVC.T)
  X      = (inc @ E) @ WprojEC.T   [m, d]
  edge   = (3+g) * E
  R      = sum_i w_i / sqrt(var(X) + eps/c_i^2)   (per row; c = [1, 2, 3+g])
  node   = (1+b)^3 * F + (X - mu(X)) * R * ec_g + b_n * ec_b

with g = vc_alpha, b = ec_alpha, w_i = (1-b)(1+b)^(2-i), b_n = sum w_i.

Device mapping: data-parallel over batch (8 cores x 1 batch element),
all matmuls in float32r (full-rate fp32 for N>=256), biases folded in via an
extra ones-row on the contraction dim, per-head softmax normalizers produced
by 32 ones-columns appended to V (PE broadcasts Z across partitions for
free), and per-head K weights zero-padded to full 128-row tiles so every
matmul operand sits at partition base 0.
"""

import os
import ml_dtypes
import numpy as np

B, M, E, D = 8, 1024, 1536, 256
H, DH = 8, 32
P = 128
EPS = 1e-5
N_CORES = 8
QW = 256  # q-slice width for attention chunks
NQ = M // QW

_COMPILED = {}


def _build(gamma: float, beta: float):
    import concourse.bass as bass
    import concourse.tile as tile
    from concourse import bacc, mybir
    from contextlib import ExitStack

    f32 = mybir.dt.float32
    bf16 = mybir.dt.bfloat16
    f32r = mybir.dt.float32r
    AF = mybir.ActivationFunctionType
    OP = mybir.AluOpType
    AX = mybir.AxisListType

    cs = [1.0, 2.0, 3.0 + gamma]
    assert all(c > 0 for c in cs), f"collapse requires positive LN scales, got {cs}"
    ws = [(1.0 - beta) * (1.0 + beta) ** 2, (1.0 - beta) * (1.0 + beta), (1.0 - beta)]
    a_f = (1.0 + beta) ** 3
    eps_i = [EPS / (c * c) for c in cs]

    nc = bacc.Bacc("TRN2", target_bir_lowering=False, debug=False)

    def din(name, shape):
        return nc.dram_tensor(name, list(shape), f32, kind="ExternalInput").ap()

    def dout(name, shape):
        return nc.dram_tensor(name, list(shape), f32, kind="ExternalOutput").ap()

    f_nat_d = din("f_nat", [P, 8, D])
    ft_d = din("ft_aug", [P, 3, M])
    wq_d = din("wq", [P, 3, 3 * P])
    wk_d = din("wk", [P, 3, 3 * P])
    wv_d = din("wv", [P, 3, D])
    wout_d = din("wout", [P, 3, D])
    wvc_d = din("wvc", [P, 2, D])
    wec_d = din("wec", [P, 3, D])
    inc_d = din("inc_nat", [P, 8, E])
    incT_bf_d = nc.dram_tensor("incT", [P, 12, M], mybir.dt.bfloat16, kind="ExternalInput").ap()
    deg_d = din("deg", [1, M])
    lnv_d = din("lnv", [4, D])
    edge_d = dout("edge_out", [P, 12, D])
    node_d = dout("node_out", [P, 8, D])

    def bcast_row(row_ap):
        return bass.AP(
            tensor=row_ap.tensor, offset=row_ap.offset, ap=[[0, P]] + list(row_ap.ap)
        )

    with tile.TileContext(nc) as tc, ExitStack() as ctx:
        consts = ctx.enter_context(tc.tile_pool(name="consts", bufs=1))
        incp = ctx.enter_context(tc.tile_pool(name="incp", bufs=2))
        attnp = ctx.enter_context(tc.tile_pool(name="attnp", bufs=1))
        stats = ctx.enter_context(tc.tile_pool(name="stats", bufs=4))
        work = ctx.enter_context(tc.tile_pool(name="work", bufs=3))
        # one [128, 1024] (2-bank) psum tag, 3 bufs = 6 banks; pv = 2 banks
        psA = ctx.enter_context(tc.tile_pool(name="psA", bufs=3, space="PSUM"))
        psB = ctx.enter_context(tc.tile_pool(name="psB", bufs=2, space="PSUM"))

        def pbig(name):
            return psA.tile([P, 1024], f32, name=name, tag="ps")

        attn_t = attnp.tile([P, 8, D], f32r, name="attn_t")

        with tc.tile_pool(name="phA", bufs=1) as phA:
            # warm the PE clock (HAM ramps to 2.4 GHz after ~3.4us of
            # activity) while the input DMAs are still in flight
            warm = phA.tile([P, P], bf16, name="warm")
            nc.vector.memset(warm, 0.0)
            wps = psB.tile([P, P], f32, name="wps", tag="pv")
            for i in range(40):
                nc.tensor.matmul(wps, lhsT=warm, rhs=warm, start=(i == 0), stop=(i == 39))

            # attention-critical loads first so compute can start ASAP
            ft_t = phA.tile([P, 3, M], f32r, name="ft_t")
            nc.sync.dma_start(out=ft_t, in_=ft_d.bitcast(f32r))
            wq_t = phA.tile([P, 3, 3 * P], f32r, name="wq_t")
            nc.sync.dma_start(out=wq_t, in_=wq_d.bitcast(f32r))
            wk_t = phA.tile([P, 3, 3 * P], f32r, name="wk_t")
            nc.sync.dma_start(out=wk_t, in_=wk_d.bitcast(f32r))
            wv_t = phA.tile([P, 3, D], f32r, name="wv_t")
            nc.sync.dma_start(out=wv_t, in_=wv_d.bitcast(f32r))
            wout_t = phA.tile([P, 3, D], f32r, name="wout_t")
            nc.sync.dma_start(out=wout_t, in_=wout_d.bitcast(f32r))

            # remaining loads (consumed later; queue behind the hot ones)
            wvc_t = consts.tile([P, 2, D], f32r, name="wvc_t")
            nc.sync.dma_start(out=wvc_t, in_=wvc_d.bitcast(f32r))
            wec_t = consts.tile([P, 3, D], f32r, name="wec_t")
            nc.sync.dma_start(out=wec_t, in_=wec_d.bitcast(f32r))
            fnat_t = consts.tile([P, 8, D], f32, name="fnat_t")
            nc.sync.dma_start(out=fnat_t, in_=f_nat_d)
            deg_t = consts.tile([1, M], f32r, name="deg_t")
            nc.sync.dma_start(out=deg_t, in_=deg_d.bitcast(f32r))

            vcg_t = consts.tile([P, D], f32, name="vcg_t")
            nc.gpsimd.dma_start(out=vcg_t, in_=bcast_row(lnv_d[0]))
            vcb_t = consts.tile([P, D], f32, name="vcb_t")
            nc.gpsimd.dma_start(out=vcb_t, in_=bcast_row(lnv_d[1]))
            ecg_t = consts.tile([P, D], f32, name="ecg_t")
            nc.gpsimd.dma_start(out=ecg_t, in_=bcast_row(lnv_d[2]))
            bpp_t = consts.tile([P, D], f32, name="bpp_t")
            nc.gpsimd.dma_start(out=bpp_t, in_=bcast_row(lnv_d[3]))

            epsv_t = consts.tile([P, 1], f32, name="epsv_t")
            nc.vector.memset(epsv_t, EPS)
            epsi_t = consts.tile([P, 3], f32, name="epsi_t")
            for i in range(3):
                nc.vector.memset(epsi_t[:, i : i + 1], eps_i[i])

            # inc streamed in thirds; incT reuses the same two slots later
            inc_c = []
            for ch in range(3):
                ic = incp.tile([P, 8, 512], f32r, name=f"inc_c{ch}", tag="incs")
                nc.sync.dma_start(out=ic, in_=inc_d[:, :, ch * 512 : (ch + 1) * 512].bitcast(f32r))
                inc_c.append(ic)

            # Q.T / K.T in the 3-tile head layout (head h at tile h//3,
            # rows 32*(h%3)..+32); unused rows of wq/wk are zero.
            qT = phA.tile([P, 3, M], f32r, name="qT")
            kT = phA.tile([P, 3, M], f32r, name="kT")
            for mt in range(3):
                for dst, wsrc in ((kT, wk_t), (qT, wq_t)):
                    ps = pbig(f"ps_qk{mt}")
                    for nh in range(2):
                        for kt in range(3):
                            nc.tensor.matmul(
                                ps[:, nh * 512 : (nh + 1) * 512],
                                lhsT=wsrc[:, kt, mt * 128 : (mt + 1) * 128],
                                rhs=ft_t[:, kt, nh * 512 : (nh + 1) * 512],
                                start=(kt == 0),
                                stop=(kt == 2),
                            )
                    nc.any.tensor_copy(out=dst[:, mt, :], in_=ps)

            # V in natural layout + 32 ones columns per head (the ones rows
            # of the PV output broadcast each head's softmax normalizer Z
            # across partitions for free).
            vaug = phA.tile([P, 8, H, 64], f32r, name="vaug")
            ones256 = phA.tile([P, D], f32, name="ones256")
            nc.vector.memset(ones256, 1.0)
            for mo in range(8):
                nc.vector.tensor_copy(
                    out=vaug[:, mo, :, 32:64],
                    in_=ones256.rearrange("p (h c) -> p h c", h=H),
                )
                ps = psB.tile([P, D], f32, name=f"ps_v{mo}", tag="pv")
                for kt in range(3):
                    nc.tensor.matmul(
                        ps,
                        lhsT=ft_t[:, kt, mo * 128 : (mo + 1) * 128],
                        rhs=wv_t[:, kt, :],
                        start=(kt == 0),
                        stop=(kt == 2),
                    )
                nc.any.tensor_copy(
                    out=vaug[:, mo, :, 0:32],
                    in_=ps.rearrange("p (h c) -> p h c", h=H),
                )

            # attnConcat.T, head h at (tile h//3, rows 32*(h%3)..+32); ones
            # row at (tile 2, row 64) feeds the bout K-augmentation.  Host
            # permutes Wout rows to match; wout rows for unused oT rows are
            # zero.  f32r init goes through DVE copies (memset can't write
            # f32r).
            oT = phA.tile([P, 3, M], f32r, name="oT")
            oinit = phA.tile([P, M], f32, name="oinit")
            nc.vector.memset(oinit, 0.0)
            nc.vector.memset(oinit[64:65, :], 1.0)
            nc.vector.tensor_copy(out=oT[:, 2, :], in_=oinit)
            nc.vector.memset(oinit[64:65, :], 0.0)
            for t in range(2):
                nc.vector.tensor_copy(out=oT[:, t, :], in_=oinit)

            # attention: q in halves of 512, scores/PV at N=512, exp over
            # [128, 1024] psum pairs
            for qh in range(2):
                for h in range(H):
                    ht, hr = h // 3, 32 * (h % 3)
                    qsl = slice(qh * 512, (qh + 1) * 512)
                    es = phA.tile([P, 8, 512], f32r, name="es", tag="es", bufs=2)
                    for kp in range(4):
                        ps = pbig(f"ps_sc{kp}")
                        for j in range(2):
                            nc.tensor.matmul(
                                ps[:, j * 512 : (j + 1) * 512],
                                lhsT=kT[hr : hr + 32, ht, (2 * kp + j) * 128 : (2 * kp + j + 1) * 128],
                                rhs=qT[hr : hr + 32, ht, qsl],
                                start=True,
                                stop=True,
                            )
                        nc.scalar.activation(
                            out=es[:, 2 * kp : 2 * kp + 2, :], in_=ps, func=AF.Exp
                        )
                    pv = psB.tile([64, 512], f32, name="pv", tag="pv")
                    for ko in range(8):
                        nc.tensor.matmul(
                            pv,
                            lhsT=vaug[:, ko, h, :],
                            rhs=es[:, ko, :],
                            start=(ko == 0),
                            stop=(ko == 7),
                        )
                    zi = stats.tile([64, 512], f32, name="zi", tag="zi")
                    nc.vector.reciprocal(out=zi[32:64, :], in_=pv[32:64, :])
                    nc.vector.tensor_mul(
                        out=oT[hr : hr + 32, ht, qsl],
                        in0=pv[0:32, :],
                        in1=zi[32:64, :],
                    )

            # attnOut = attnConcat @ Wout.T + bout   [m, d] natural
            for mp in range(4):
                ps = pbig(f"ps_ao{mp}")
                for mi in range(2):
                    mt = 2 * mp + mi
                    for kt in range(3):
                        nc.tensor.matmul(
                            ps[:, mi * 512 : mi * 512 + D],
                            lhsT=oT[:, kt, mt * 128 : (mt + 1) * 128],
                            rhs=wout_t[:, kt, :],
                            start=(kt == 0),
                            stop=(kt == 2),
                        )
                nc.any.tensor_copy(
                    out=attn_t[:, 2 * mp : 2 * mp + 2, :],
                    in_=ps.rearrange("p (a d) -> p a d", a=2)[:, :, :D],
                )

        # ---- S.T = attn.T @ inc  [d=256 -> 2 ptiles, e=1536] ----
        with tc.tile_pool(name="phB", bufs=1) as phB:
            st = phB.tile([P, 2, E], f32, name="st")
            est = phB.tile([P, 2, E], f32r, name="est")
            zpart = stats.tile([P, 2, 3], f32, name="zpart", tag="zp")
            for ch in range(3):
                for dt in range(2):
                    if dt == 0:
                        ps = pbig(f"ps_st{ch}{dt}")
                    else:
                        ps = psB.tile([P, 512], f32, name=f"ps_st{ch}{dt}", tag="pv")
                    for kt in range(8):
                        nc.tensor.matmul(
                            ps[:, :512],
                            lhsT=attn_t[:, kt, dt * 128 : (dt + 1) * 128],
                            rhs=inc_c[ch][:, kt, :],
                            start=(kt == 0),
                            stop=(kt == 7),
                        )
                    sl = slice(ch * 512, (ch + 1) * 512)
                    nc.any.tensor_copy(out=st[:, dt, sl], in_=ps[:, :512])
                    nc.scalar.activation(
                        out=est[:, dt, sl],
                        in_=ps[:, :512],
                        func=AF.Exp,
                        accum_out=zpart[:, dt, ch : ch + 1],
                    )
            zsum = stats.tile([P, 2], f32, name="zsum", tag="zs")
            nc.vector.reduce_sum(out=zsum, in_=zpart, axis=AX.X)
            winv = stats.tile([P, 2], f32, name="winv", tag="wi")
            nc.vector.reciprocal(out=winv, in_=zsum)
            for dt in range(2):
                nc.vector.scalar_tensor_tensor(
                    out=est[:, dt, :],
                    in0=est[:, dt, :],
                    scalar=winv[:, dt : dt + 1],
                    in1=st[:, dt, :],
                    op0=OP.mult,
                    op1=OP.mult,
                )

            # E_proj per e-tile; LayerNorm kept PRE-affine (norm only).  The
            # vc-LN affine is folded into wec (g) + deg row (b) for the node
            # path, and into the host-scaled (c_e*g, c_e*b) broadcast vectors
            # for the edge output.  Scalar chains are batched across tiles.
            ecore = phB.tile([P, 12, D], bf16, name="ecore")
            ep_all = phB.tile([P, 12, D], f32, name="ep_all")
            mv_all = stats.tile([P, 12, 2], f32, name="mv_all", tag="mva", bufs=1)
            for ep in range(6):
                psp = pbig(f"ps_ep{ep}")
                for sub in range(2):
                    et = 2 * ep + sub
                    ps = psp[:, sub * 512 : sub * 512 + D]
                    for kt in range(2):
                        nc.tensor.matmul(
                            ps,
                            lhsT=est[:, kt, et * 128 : (et + 1) * 128],
                            rhs=wvc_t[:, kt, :],
                            start=(kt == 0),
                            stop=(kt == 1),
                        )
                    bst = stats.tile([P, 6], f32, name="bst", tag="bst")
                    nc.vector.bn_stats(out=bst, in_=ps)
                    nc.vector.bn_aggr(out=mv_all[:, 2 * ep + sub, :], in_=bst)
                nc.any.tensor_copy(
                    out=ep_all[:, 2 * ep : 2 * ep + 2, :],
                    in_=psp.rearrange("p (a d) -> p a d", a=2)[:, :, :D],
                )
            rstd_all = stats.tile([P, 12], f32, name="rstd_all", tag="rsa", bufs=1)
            for half in range(2):
                hs = slice(6 * half, 6 * half + 6)
                sq_h = stats.tile([P, 6], f32, name="sq_h", tag="sqa", bufs=2)
                nc.scalar.activation(
                    out=sq_h, in_=mv_all[:, hs, 1], func=AF.Sqrt, bias=epsv_t
                )
                nc.vector.reciprocal(out=rstd_all[:, hs], in_=sq_h)
                for et in range(6 * half, 6 * half + 6):
                    etmp = work.tile([P, D], f32, name="etmp", tag="etmp")
                    nc.vector.tensor_scalar(
                        out=etmp,
                        in0=ep_all[:, et, :],
                        scalar1=mv_all[:, et, 0:1],
                        scalar2=rstd_all[:, et : et + 1],
                        op0=OP.subtract,
                        op1=OP.mult,
                    )
                    nc.vector.tensor_copy(out=ecore[:, et, :], in_=etmp)
                    eo = work.tile([P, D], f32, name="eo", tag="eo")
                    nc.gpsimd.tensor_mul(out=eo, in0=etmp, in1=vcg_t)
                    nc.gpsimd.tensor_add(out=eo, in0=eo, in1=vcb_t)
                    nc.sync.dma_start(out=edge_d[:, et, :], in_=eo)

            # X.T = norm.T @ incT  [d=256 -> 2 ptiles, m=1024], incT streamed
            # in thirds through the inc slots; one 2-bank psum per dt
            nin = phB.tile([P, 2, M], f32r, name="nin")
            ps_ni = [pbig(f"ps_ni{dt}") for dt in range(2)]
            for tc_i in range(3):
                it = incp.tile([P, 4, M], bf16, name=f"incT_c{tc_i}", tag="incs")
                nc.sync.dma_start(
                    out=it, in_=incT_bf_d[:, tc_i * 4 : (tc_i + 1) * 4, :]
                )
                for dt in range(2):
                    for nh in range(2):
                        for kt in range(4):
                            nc.tensor.matmul(
                                ps_ni[dt][:, nh * 512 : (nh + 1) * 512],
                                lhsT=ecore[:, tc_i * 4 + kt, dt * 128 : (dt + 1) * 128],
                                rhs=it[:, kt, nh * 512 : (nh + 1) * 512],
                                start=(tc_i == 0 and kt == 0),
                                stop=(tc_i == 2 and kt == 3),
                            )
            for dt in range(2):
                nc.any.tensor_copy(out=nin[:, dt, :], in_=ps_ni[dt])

            # X = nin.T @ (diag(g) WecT) + deg (x) bWec, then the 3-eps LN
            # combine; scalar chains batched across the 8 m-tiles.
            np_all = phB.tile([P, 8, D], f32, name="np_all")
            mvn_all = stats.tile([P, 8, 2], f32, name="mvn_all", tag="mvn", bufs=1)
            for mt in range(8):
                ps = psB.tile([P, D], f32, name=f"ps_np{mt}", tag="pv")
                for kt in range(2):
                    nc.tensor.matmul(
                        ps,
                        lhsT=nin[:, kt, mt * 128 : (mt + 1) * 128],
                        rhs=wec_t[:, kt, :],
                        start=(kt == 0),
                        stop=False,
                    )
                nc.tensor.matmul(
                    ps,
                    lhsT=deg_t[0:1, mt * 128 : (mt + 1) * 128],
                    rhs=wec_t[0:1, 2, :],
                    start=False,
                    stop=True,
                )
                bst = stats.tile([P, 6], f32, name="bstn", tag="bst")
                nc.vector.bn_stats(out=bst, in_=ps)
                nc.vector.bn_aggr(out=mvn_all[:, mt, :], in_=bst)
                nc.any.tensor_copy(out=np_all[:, mt, :], in_=ps)
            R_all = stats.tile([P, 8], f32, name="R_all", tag="Ra", bufs=1)
            R_all = stats.tile([P, 8], f32, name="R_all", tag="Ra", bufs=1)
            R_all = stats.tile([P, 8], f32, name="R_all", tag="Ra", bufs=1)
            for i in range(3):
                sqn = stats.tile([P, 8], f32, name="sqn_all", tag="sqn", bufs=3)
                nc.scalar.activation(
                    out=sqn, in_=mvn_all[:, :, 1], func=AF.Sqrt, bias=epsi_t[:, i : i + 1]
                )
                rin = stats.tile([P, 8], f32, name="rin_all", tag="rin", bufs=3)
                nc.vector.reciprocal(out=rin, in_=sqn)
                if i == 0:
                    nc.vector.tensor_scalar_mul(R_all, rin, ws[0])
                else:
                    nc.vector.scalar_tensor_tensor(
                        out=R_all, in0=rin, scalar=ws[i], in1=R_all, op0=OP.mult, op1=OP.add
                    )
            for mt in range(8):
                t = work.tile([P, D], f32, name="t", tag="t")
                nc.vector.tensor_scalar(
                    out=t,
                    in0=np_all[:, mt, :],
                    scalar1=mvn_all[:, mt, 0:1],
                    scalar2=R_all[:, mt : mt + 1],
                    op0=OP.subtract,
                    op1=OP.mult,
                )
                nc.gpsimd.tensor_mul(out=t, in0=t, in1=ecg_t)
                nc.gpsimd.tensor_add(out=t, in0=t, in1=bpp_t)
                no = work.tile([P, D], f32, name="no", tag="no")
                nc.vector.scalar_tensor_tensor(
                    out=no, in0=fnat_t[:, mt, :], scalar=a_f, in1=t, op0=OP.mult, op1=OP.add
                )
                nc.sync.dma_start(out=node_d[:, mt, :], in_=no)

    nc.compile()
    return nc


def _tile3(a, kt):
    """[kt*128, X] -> [128, kt, X] with row index = k*128 + p."""
    return np.ascontiguousarray(a.reshape(kt, P, -1).transpose(1, 0, 2))


def host_prep(inputs):
    f32 = np.float32
    F = np.asarray(inputs["features"], f32)
    inc = np.asarray(inputs["inc_mat"], f32)
    gamma = float(np.asarray(inputs["vc_alpha"]))
    beta = float(np.asarray(inputs["ec_alpha"]))

    inv_s = 1.0 / np.sqrt(np.float32(DH))
    WinT = np.asarray(inputs["vc_Win"], f32).T  # [256, 768]
    binv = np.asarray(inputs["vc_bin"], f32)

    # Q/K heads in a 3-tile layout: head h -> col 128*(h//3) + 32*(h%3),
    # so every 32-row matmul operand sits at partition base 0/32/64.
    wq = np.zeros((3 * P, 3 * P), f32)
    wk = np.zeros((3 * P, 3 * P), f32)
    for h in range(H):
        off = P * (h // 3) + 32 * (h % 3)
        wq[:D, off : off + 32] = WinT[:, 32 * h : 32 * (h + 1)] * inv_s
        wq[D, off : off + 32] = binv[32 * h : 32 * (h + 1)] * inv_s
        wk[:D, off : off + 32] = WinT[:, D + 32 * h : D + 32 * (h + 1)]
        wk[D, off : off + 32] = binv[D + 32 * h : D + 32 * (h + 1)]

    wv = np.zeros((3 * P, D), f32)
    wv[:D] = WinT[:, 2 * D :]
    wv[D] = binv[2 * D :]

    # Wout rows permuted to match the oT layout (head h at row
    # 128*(h//3) + 32*(h%3)), bout at row 2*128+64.
    WoutT = np.asarray(inputs["vc_Wout"], f32).T  # [256, 256]
    wout = np.zeros((3 * P, D), f32)
    for h in range(H):
        r = P * (h // 3) + 32 * (h % 3)
        wout[r : r + 32] = WoutT[32 * h : 32 * (h + 1)]
    wout[2 * P + 64] = np.asarray(inputs["vc_bout"], f32)

    wvc = _tile3(np.ascontiguousarray(np.asarray(inputs["vc_Wproj"], f32).T), 2)

    # The vc LayerNorm affine (g, b) is folded into the edge->node path:
    #   inc @ (norm*g + 1{x}b) @ WecT = inc @ norm @ (diag(g) WecT) + deg (x) (b @ WecT)
    # so the device only materializes the pre-affine norm; the deg rank-1
    # term rides an extra contraction row (host-computed degree vector).
    vc_g = np.asarray(inputs["vc_ln_g"], f32)
    vc_b = np.asarray(inputs["vc_ln_b"], f32)
    WecT = np.asarray(inputs["ec_Wproj"], f32).T  # [256, 256]
    wec = np.zeros((3 * P, D), f32)
    wec[:D] = WecT * vc_g[:, None]
    wec[D] = vc_b @ WecT

    gamma_ce = 3.0 + gamma
    ws_sum = (1 - beta) * ((1 + beta) ** 2 + (1 + beta) + 1)
    lnv = np.stack(
        [
            gamma_ce * vc_g,
            gamma_ce * vc_b,
            np.asarray(inputs["ec_ln_g"], f32),
            (ws_sum * np.asarray(inputs["ec_ln_b"], f32)).astype(f32),
        ]
    )

    shared = {
        "wq": _tile3(wq, 3),
        "wk": _tile3(wk, 3),
        "wv": _tile3(wv, 3),
        "wout": _tile3(wout, 3),
        "wvc": wvc,
        "wec": _tile3(wec, 3),
        "lnv": np.ascontiguousarray(lnv),
    }

    in_maps = []
    for c in range(N_CORES):
        ft = np.zeros((3 * P, M), f32)
        ft[:D] = F[c].T
        ft[D] = 1.0
        m = dict(shared)
        m["f_nat"] = _tile3(F[c], 8)
        m["ft_aug"] = _tile3(ft, 3)
        m["inc_nat"] = _tile3(inc[c], 8)
        m["incT"] = _tile3(np.ascontiguousarray(inc[c].T), 12).astype(ml_dtypes.bfloat16)
        m["deg"] = np.ascontiguousarray(inc[c].sum(axis=1, dtype=f32).reshape(1, M))
        in_maps.append(m)
    return in_maps, gamma, beta


LAST_RESULT = None


def kernel(**inputs):
    global LAST_RESULT
    from concourse.bass_utils import run_bass_kernel_spmd

    in_maps, gamma, beta = host_prep(inputs)

    key = (gamma, beta)
    if key not in _COMPILED:
        _COMPILED[key] = _build(gamma, beta)
    nc = _COMPILED[key]

    trace = bool(int(os.environ.get("KERNEL_TRACE", "0")))
    res = run_bass_kernel_spmd(
        nc, in_maps, core_ids=list(range(N_CORES)), trace=trace
    )
    LAST_RESULT = res

    node = np.stack(
        [r["node_out"].transpose(1, 0, 2).reshape(M, D) for r in res.results]
    )
    edge = np.stack(
        [r["edge_out"].transpose(1, 0, 2).reshape(E, D) for r in res.results]
    )
    return node, edge, np.asarray(inputs["inc_mat"], np.float32)
